# revision 1
# baseline (speedup 1.0000x reference)
"""TRN2 Bass/Tile kernel for nn_ClassifierHetero (batched heterograph classifier).

In the reference forward, the HeteroGraphConv stack is dead code (its outputs
are deleted and never read): the module output depends only on the per-graph
means of the ORIGINAL node features, concatenated to [B, 4], followed by a
3-layer MLP -> [B, 10].

Sharding (per the hint): data-parallel over graphs — 8 graphs per core x 8
cores; the tiny MLP weights are replicated. The gid arrays are sorted, so
each graph's node rows are a contiguous slice; the host packs each graph's
rows (zero-padded to a fixed capacity) into a [128, W] layout where graph g
owns partitions [16g, 16g+16). On device:
  1. vector-engine free-dim sums -> 16 partial sums per (graph, feature)
  2. scale by 1/max(count,1) (pre-expanded per partition) on DVE
  3. one PE matmul against a 0/1 selector collapses partials -> [4, 8] means
  4. 3-layer MLP: 4 PE matmuls; bias+relu fused into single DVE tensor_scalar
     ops (per-partition bias columns); the last layer is computed transposed
     ([NCLS, G], classes on partitions) so bc3 is a per-partition bias too,
     and the host transposes while unsharding.

Constraints of the bass2jax/neuronxcc codegen path shaped the program:
  - only ONE sync-wait command per instruction: each engine absorbs each
    DMA-completion wait exactly once via cheap staging copies, both operands
    of every matmul come from the DVE, and the Tile kernel-tail drain is
    re-emitted as a chain of single-wait drains (see _patch_tile_tail);
  - engine APs must start at partition 0/32/64;
  - DMAs are split across the two HWDGE rings (SP + ACT) plus one gpsimd
    SWDGE transfer so the transfers overlap and reduces start as data lands.

Self-contained: all shapes/constants hardcoded from the problem spec.
"""

import numpy as np

# --- problem constants (hardcoded from the spec) ---
B = 64            # graphs in the batch
NCORES = 8
G = B // NCORES   # graphs per core
HID = 128
NCLS = 10
NSUB = 16         # SBUF partitions per graph: partition p = g*NSUB + s
P_FULL = G * NSUB  # = 128

# Default per-graph column widths (capacity per graph = NSUB * W).
# Graph sizes are ~Binomial(N, 1/64): comp ~1562+-39, port ~6250+-78,
# net ~2344+-48 -> defaults give >5 sigma of margin; widths auto-escalate
# (with recompile) if an input ever exceeds them.
W_C0, W_P0, W_N0 = 64, 256, 96

# params buffer column layout ([128, PA], per core):
#   Wc2 | Wc3 | Sel | recfull | bc1 | bc2 | bc3col
_WC3_OFF = HID                          # 128..138
_SEL_OFF = HID + NCLS                   # 138..146
_RECF_OFF = _SEL_OFF + G                # 146..150
_BC1_COL = _RECF_OFF + 4                # 150
_BC2_COL = _BC1_COL + 1                 # 151
_BC3_COL = _BC2_COL + 1                 # 152 (partitions 0..9 hold bc3)
PA = _BC3_COL + 1                       # 153

_NC_CACHE: dict = {}


def _round_up(x: int, m: int) -> int:
    return -(-x // m) * m


def _widths(cnt_c, cnt_p, cnt_n):
    def w_for(maxcnt, w0):
        need = _round_up(_round_up(int(maxcnt), NSUB) // NSUB, 16)
        return max(w0, need)

    return (
        w_for(cnt_c.max(), W_C0),
        w_for(cnt_p.max(), W_P0),
        w_for(cnt_n.max(), W_N0),
    )


def _patch_tile_tail():
    """The neuronxcc codegen used by the bass2jax path allows only ONE
    sync-wait command per instruction, but TileContext's kernel-tail drain
    waits on every live semaphore at once. Re-emit that tail as a chain of
    single-wait drains (one per logical processor of the global clock)."""
    import concourse.tile as tile

    if getattr(tile.TileContext, "_single_wait_tail", False):
        return
    from concourse.vector_clock import ScopedClock, VectorClock

    def _drain_and_barrier(self, tick_clock, wait_clock):
        nc = self.nc
        gc = tick_clock.global_clock
        n = len(gc)
        for proc in range(n):
            t = gc[proc]
            if t <= 0:
                continue
            sub = VectorClock([0] * n)
            sub.require_at_least(proc, t)
            d = nc.sync.drain(fusable=False)
            wait_clock.add_sem_waits(d.ins, ScopedClock({None: sub}))
        nc.sync.drain(fusable=False)
        nc.all_engine_barrier()
        assert self.sems is not None
        popped = nc._tile_sem_poison_stack.pop()
        assert popped is self._sem_poison
        nc.clear_and_free_semaphores(list(self.sems.allocated().values()))
        nc.all_engine_barrier()

    tile.TileContext._drain_and_barrier = _drain_and_barrier
    tile.TileContext._single_wait_tail = True


def _build_nc(wc: int, wp: int, wn: int):
    import concourse.bass as bass
    import concourse.mybir as mybir
    import concourse.tile as tile
    from concourse.tile import add_dep_helper

    _patch_tile_tail()
    f32 = mybir.dt.float32
    X = mybir.AxisListType.X
    ADD = mybir.AluOpType.add
    MAX = mybir.AluOpType.max
    nc = bass.Bass()

    a_ext = nc.declare_dram_parameter("pa", [P_FULL, PA], f32, isOutput=False)
    q_ext = nc.declare_dram_parameter("qw1", [4, HID], f32, isOutput=False)
    c_ext = nc.declare_dram_parameter("dcn", [P_FULL, wc + wn], f32, isOutput=False)
    p0_ext = nc.declare_dram_parameter("dp0", [P_FULL, wp], f32, isOutput=False)
    p1_ext = nc.declare_dram_parameter("dp1", [P_FULL, wp], f32, isOutput=False)
    out_ext = nc.declare_dram_parameter("out", [NCLS, G], f32, isOutput=True)

    # Raw (non-Tile) SBUF buffers for the inputs. Their DMAs are issued from
    # a plain block that runs during the fixed framework preamble, split
    # across the two HWDGE rings (SP + ACT), params first (their consumers
    # unblock the most work). One semaphore per transfer lets each consumer
    # start as soon as ITS data has landed; NRT zeroes semaphores at
    # execution start. No gpsimd/SWDGE transfer: its end-of-block drain
    # would stall the block-exit barrier until the transfer lands.
    At = nc.alloc_sbuf_tensor("At", [P_FULL, PA], f32)
    Qt = nc.alloc_sbuf_tensor("Qt", [4, HID], f32)
    Ct = nc.alloc_sbuf_tensor("Ct", [P_FULL, wc + wn], f32)
    P0t = nc.alloc_sbuf_tensor("P0t", [P_FULL, wp], f32)
    P1t = nc.alloc_sbuf_tensor("P1t", [P_FULL, wp], f32)
    sems = {n: nc.alloc_semaphore(f"dma_{n}") for n in ("a", "q", "c", "p0", "p1")}

    with nc.Block(no_gpsimd_drain=True) as blk:

        @blk.sync
        def _(s):
            s.dma_start(out=Qt[:], in_=q_ext[:]).then_inc(sems["q"], 16)
            s.dma_start(out=P0t[:], in_=p0_ext[:]).then_inc(sems["p0"], 16)

        @blk.scalar
        def _(s):
            s.dma_start(out=At[:], in_=a_ext[:]).then_inc(sems["a"], 16)
            s.dma_start(out=P1t[:], in_=p1_ext[:]).then_inc(sems["p1"], 16)

        @blk.gpsimd
        def _(s):
            s.dma_start(out=Ct[:], in_=c_ext[:]).then_inc(sems["c"], 16)

    gates = []

    def gate_for(sem, engine=None):
        # emitted with wait value 0 so the Tile scheduling sim (which never
        # executes the pre-block's increments) doesn't deadlock; the real
        # value (16 = one DMA transfer) is patched in post-schedule.
        g = (engine or nc.vector).wait_ge(sem, 0)
        gates.append(g)
        return g

    with tile.TileContext(nc) as tc:
        with (
            tc.tile_pool(name="sbuf", bufs=1) as pool,
            tc.tile_pool(name="psum", bufs=1, space="PSUM") as psum,
        ):
            sel_t = pool.tile([P_FULL, G], f32)
            recf_t = pool.tile([P_FULL, 4], f32)
            w1_t = pool.tile([4, HID], f32)
            wc2_t = pool.tile([P_FULL, HID], f32)
            wc3_t = pool.tile([P_FULL, NCLS], f32)
            S = pool.tile([P_FULL, 4], f32)
            S2 = pool.tile([P_FULL, 4], f32)
            hgT = pool.tile([4, G], f32)
            h1 = pool.tile([HID, G], f32)
            h2 = pool.tile([HID, G], f32)
            otT = pool.tile([NCLS, G], f32)
            ps_hg = psum.tile([4, G], f32)
            ps_h1 = psum.tile([HID, G], f32)
            ps_h2 = psum.tile([HID, G], f32)
            ps_oT = psum.tile([NCLS, G], f32)

            dep = []  # (consumer, gate) pairs

            # --- DVE: staging + reductions, gated per transfer -----------
            ga = gate_for(sems["a"])
            r = nc.vector.tensor_copy(sel_t[:], At[:, _SEL_OFF : _SEL_OFF + G])
            dep.append((r, ga))
            r = nc.vector.tensor_copy(recf_t[:], At[:, _RECF_OFF : _RECF_OFF + 4])
            dep.append((r, ga))
            gp0 = gate_for(sems["p0"])
            r = nc.vector.reduce_sum(S[:, 1:2], P0t[:], axis=X)
            dep.append((r, gp0))
            gp1 = gate_for(sems["p1"])
            r = nc.vector.reduce_sum(S[:, 2:3], P1t[:], axis=X)
            dep.append((r, gp1))
            gc_ = gate_for(sems["c"])
            r = nc.vector.reduce_sum(S[:, 0:1], Ct[:, 0:wc], axis=X)
            dep.append((r, gc_))
            r = nc.vector.reduce_sum(S[:, 3:4], Ct[:, wc : wc + wn], axis=X)
            dep.append((r, gc_))
            # scale partials by 1/max(count,1) (expanded per partition)
            nc.vector.tensor_mul(S2[:], S[:], recf_t[:])

            # collapse 16 scaled partials per graph -> means [4, G]
            nc.tensor.matmul(
                ps_hg[:], lhsT=S2[:], rhs=sel_t[:], start=True, stop=True
            )
            gq = gate_for(sems["q"])
            r = nc.vector.tensor_copy(w1_t[:], Qt[0:4, 0:HID])
            dep.append((r, gq))
            nc.vector.tensor_copy(hgT[:], ps_hg[:])

            # layer 1: h1T = relu(Wc1.T @ hgT + bc1)
            nc.tensor.matmul(
                ps_h1[:], lhsT=w1_t[:], rhs=hgT[:], start=True, stop=True
            )
            r = nc.vector.tensor_copy(wc2_t[:], At[:, 0:HID])
            dep.append((r, ga))
            r = nc.vector.tensor_scalar(
                h1[:], ps_h1[:], At[:, _BC1_COL : _BC1_COL + 1], 0.0,
                op0=ADD, op1=MAX,
            )
            dep.append((r, ga))
            # layer 2: h2T = relu(Wc2.T @ h1T + bc2)
            nc.tensor.matmul(
                ps_h2[:], lhsT=wc2_t[:], rhs=h1[:], start=True, stop=True
            )
            r = nc.vector.tensor_copy(wc3_t[:], At[:, _WC3_OFF : _WC3_OFF + NCLS])
            dep.append((r, ga))
            r = nc.vector.tensor_scalar(
                h2[:], ps_h2[:], At[:, _BC2_COL : _BC2_COL + 1], 0.0,
                op0=ADD, op1=MAX,
            )
            dep.append((r, ga))
            # layer 3 (transposed): outT = Wc3.T @ h2T + bc3  [NCLS, G]
            nc.tensor.matmul(
                ps_oT[:], lhsT=wc3_t[:], rhs=h2[:], start=True, stop=True
            )
            r = nc.vector.tensor_scalar(
                otT[:], ps_oT[:], At[0:NCLS, _BC3_COL : _BC3_COL + 1], None,
                op0=ADD,
            )
            dep.append((r, ga))
            nc.sync.dma_start(out=out_ext[:], in_=otT[:])

            for consumer, g in dep:
                add_dep_helper(
                    consumer.ins, g.ins, False, "raw input read after DMA gate"
                )

    for g in gates:
        g.ins.sync_info.on_wait[0].wait_value = 16
    return nc


def _get_nc(wc: int, wp: int, wn: int):
    key = (wc, wp, wn)
    if key not in _NC_CACHE:
        _NC_CACHE[key] = _build_nc(wc, wp, wn)
    return _NC_CACHE[key]


def _pack_col(out, col_off, h, col, bounds, width):
    """Pack one (node type, feature col) into out[:, :, col_off:col_off+width]."""
    cap = NSUB * width
    for b in range(B):
        m, g = divmod(b, G)
        s, e = int(bounds[b]), int(bounds[b + 1])
        n = e - s
        if n == 0:
            continue
        buf = np.zeros(cap, np.float32)
        buf[:n] = h[s:e, col]
        p0 = g * NSUB
        out[m, p0 : p0 + NSUB, col_off : col_off + width] = buf.reshape(NSUB, width)


def _prepare(inputs):
    h_comp = np.ascontiguousarray(np.asarray(inputs["h_comp"], dtype=np.float32))
    h_port = np.ascontiguousarray(np.asarray(inputs["h_port"], dtype=np.float32))
    h_net = np.ascontiguousarray(np.asarray(inputs["h_net"], dtype=np.float32))
    gid_c = np.asarray(inputs["gid_comp"])
    gid_p = np.asarray(inputs["gid_port"])
    gid_n = np.asarray(inputs["gid_net"])

    edges = np.arange(B + 1)
    bc = np.searchsorted(gid_c, edges)
    bp = np.searchsorted(gid_p, edges)
    bn = np.searchsorted(gid_n, edges)
    cnt_c = np.diff(bc)
    cnt_p = np.diff(bp)
    cnt_n = np.diff(bn)

    wc, wp, wn = _widths(cnt_c, cnt_p, cnt_n)

    Wc1 = np.asarray(inputs["Wc1"], dtype=np.float32)
    bc1 = np.asarray(inputs["bc1"], dtype=np.float32)
    Wc2 = np.asarray(inputs["Wc2"], dtype=np.float32)
    bc2 = np.asarray(inputs["bc2"], dtype=np.float32)
    Wc3 = np.asarray(inputs["Wc3"], dtype=np.float32)
    bc3 = np.asarray(inputs["bc3"], dtype=np.float32)

    # rec[j, b] = 1/max(count_type(j)[b], 1)
    rec = np.empty((4, B), np.float32)
    rec[0] = 1.0 / np.maximum(cnt_c, 1)
    rec[1] = 1.0 / np.maximum(cnt_p, 1)
    rec[2] = rec[1]
    rec[3] = 1.0 / np.maximum(cnt_n, 1)

    sel = (np.arange(P_FULL)[:, None] // NSUB == np.arange(G)[None, :]).astype(
        np.float32
    )

    A = np.zeros((NCORES, P_FULL, PA), np.float32)
    A[:, :, 0:HID] = Wc2
    A[:, :, _WC3_OFF : _WC3_OFF + NCLS] = Wc3
    A[:, :, _SEL_OFF : _SEL_OFF + G] = sel
    for m in range(NCORES):
        g_of_p = m * G + np.arange(P_FULL) // NSUB
        A[m, :, _RECF_OFF : _RECF_OFF + 4] = rec[:, g_of_p].T
    A[:, :, _BC1_COL] = bc1
    A[:, :, _BC2_COL] = bc2
    A[:, 0:NCLS, _BC3_COL] = bc3

    C = np.zeros((NCORES, P_FULL, wc + wn), np.float32)
    P0 = np.zeros((NCORES, P_FULL, wp), np.float32)
    P1 = np.zeros((NCORES, P_FULL, wp), np.float32)
    _pack_col(C, 0, h_comp, 0, bc, wc)
    _pack_col(C, wc, h_net, 0, bn, wn)
    _pack_col(P0, 0, h_port, 0, bp, wp)
    _pack_col(P1, 0, h_port, 1, bp, wp)

    Qw1 = np.ascontiguousarray(Wc1)

    in_maps = [
        {"pa": A[m], "qw1": Qw1, "dcn": C[m], "dp0": P0[m], "dp1": P1[m]}
        for m in range(NCORES)
    ]
    return (wc, wp, wn), in_maps


def _run(inputs, trace=False, **kwargs):
    from concourse.bass_utils import run_bass_kernel_spmd

    (wc, wp, wn), in_maps = _prepare(inputs)
    nc = _get_nc(wc, wp, wn)
    res = run_bass_kernel_spmd(
        nc, in_maps, list(range(NCORES)), trace=trace, **kwargs
    )
    # per-core output is [NCLS, G] (classes on partitions) — transpose back
    out = np.concatenate(
        [res.results[m]["out"].T for m in range(NCORES)], axis=0
    ).astype(np.float32)
    return out, res


def kernel(**inputs) -> np.ndarray:
    out, _ = _run(inputs, trace=False)
    return out


def run_traced(inputs, **kwargs):
    out, res = _run(inputs, trace=True, **kwargs)
    return out, res


def simulate_numpy(**inputs):
    """Numpy emulation of the device program (for fast logic validation)."""
    (wc, wp, wn), in_maps = _prepare(inputs)
    outs = []
    for m in range(NCORES):
        im = in_maps[m]
        A, Qw1, C, P0, P1 = (
            im["pa"], im["qw1"], im["dcn"], im["dp0"], im["dp1"],
        )
        S = np.zeros((P_FULL, 4), np.float32)
        S[:, 0] = C[:, 0:wc].sum(1)
        S[:, 1] = P0.sum(1)
        S[:, 2] = P1.sum(1)
        S[:, 3] = C[:, wc : wc + wn].sum(1)
        S2 = S * A[:, _RECF_OFF : _RECF_OFF + 4]
        sel = A[:, _SEL_OFF : _SEL_OFF + G]
        hgT = S2.T @ sel                      # [4, G] means
        h1 = np.maximum(Qw1.T @ hgT + A[:, _BC1_COL : _BC1_COL + 1], 0.0)
        h2 = np.maximum(A[:, 0:HID].T @ h1 + A[:, _BC2_COL : _BC2_COL + 1], 0.0)
        oT = (A[:, _WC3_OFF : _WC3_OFF + NCLS].T @ h2
              + A[0:NCLS, _BC3_COL : _BC3_COL + 1])
        outs.append(oT.T)
    return np.concatenate(outs, 0).astype(np.float32)

